# revision 23
# baseline (speedup 1.0000x reference)
"""CRvNN-style masked recurrence kernel for Trainium2 (8 NeuronCores).

Problem shapes (hardcoded): N=64, S=512, D=512, C=1024.
Sharding: data-parallel over batch N across 8 cores (8 examples/core),
weights replicated. The sequential S-recurrence runs on-device in a
hardware loop with a feature-major ("transposed") layout so that all
vector/scalar work uses full 128-partition tiles.

Per step (per core, batch B=8):
  interT = gelu(W1_top.T-tiles @ htT + Z'_s)        [C feature-major]
  contsT = W2-tiles @ interT + b2                   [4D feature-major]
  gates  = sigmoid(contsT[:3D]); parent = contsT[3D:]
  t      = f1*ht + f2*x_s + i*parent
  htT    = LN(t)  (stats via ones-matmul over partitions, PE broadcast back)
Z' = x @ W1_bot + b1 is hoisted out of the loop (computed in bulk).
"""

import numpy as np
import ml_dtypes

import concourse.bass as bass
from concourse import bacc
import concourse.mybir as mybir
import concourse.tile as tile
from concourse.bass import ds

F32 = mybir.dt.float32
BF16 = mybir.dt.bfloat16
FP16 = mybir.dt.float16

N, S, D, C = 64, 512, 512, 1024
NCORES = 8
B = N // NCORES          # examples per core = 8
TOK = B * S              # tokens per core = 4096
NTT = TOK // 128         # token tiles = 32
DC = D // 128            # = 4 d-chunks
CC = C // 128            # = 8 c-chunks
FC = (4 * D) // 128      # = 16 contents chunks
LN_EPS = 1e-5
AF = mybir.ActivationFunctionType
OP = mybir.AluOpType


def build(n_steps=S, unroll=1):
    nc = bacc.Bacc("TRN2", target_bir_lowering=False, debug=False, enable_asserts=False)

    # ---- DRAM I/O (per-core) ----
    # seq columns 0:D = sequence (bf16); column D = input mask (bf16)
    seq_d = nc.dram_tensor("seq", [TOK, D + 1], BF16, kind="ExternalInput")
    wi_d = nc.dram_tensor("wi", [128, DC, D], BF16, kind="ExternalInput")
    w1t_d = nc.dram_tensor("w1t", [128, DC, C], BF16, kind="ExternalInput")
    w1b_d = nc.dram_tensor("w1b", [128, DC, C], BF16, kind="ExternalInput")
    w2_d = nc.dram_tensor("w2", [128, CC, 4 * D], BF16, kind="ExternalInput")
    b1t_d = nc.dram_tensor("b1t", [128, CC], F32, kind="ExternalInput")
    b2f_d = nc.dram_tensor("b2f", [128, FC, B], F32, kind="ExternalInput")
    gT_d = nc.dram_tensor("gT", [128, DC], F32, kind="ExternalInput")
    bT_d = nc.dram_tensor("bT", [128, DC], F32, kind="ExternalInput")
    grow_d = nc.dram_tensor("grow", [1, D], BF16, kind="ExternalInput")
    brow_d = nc.dram_tensor("brow", [1, D], BF16, kind="ExternalInput")
    birow_d = nc.dram_tensor("birow", [1, D], BF16, kind="ExternalInput")
    h0_d = nc.dram_tensor("h0", [128, DC, B], BF16, kind="ExternalInput")
    idb_d = nc.dram_tensor("idb", [128, 128], BF16, kind="ExternalInput")
    idh_d = nc.dram_tensor("idh", [128, 128], FP16, kind="ExternalInput")
    out_d = nc.dram_tensor("out", [TOK, D], FP16, kind="ExternalOutput")

    with tile.TileContext(nc) as tc:
        with tc.tile_pool(name="persist", bufs=1) as pp:
            # persistent SBUF tensors
            wi_sb = pp.tile([128, DC, D], BF16)
            w1t_sb = pp.tile([128, DC, C], BF16)
            w1b_sb = pp.tile([128, DC, C], BF16)
            w2_sb = pp.tile([128, CC, 4 * D], BF16)
            b1t_sb = pp.tile([128, CC], F32)
            b2f_sb = pp.tile([128, FC, B], F32)
            gT_sb = pp.tile([128, DC], F32)
            bT_sb = pp.tile([128, DC], F32)
            gbc_sb = pp.tile([128, D], BF16)   # ln_g broadcast along partitions
            bbc_sb = pp.tile([128, D], BF16)   # ln_b broadcast
            bibc_sb = pp.tile([128, D], BF16)  # b_init broadcast
            idb_sb = pp.tile([128, 128], BF16)
            idh_sb = pp.tile([128, 128], FP16)
            ones_b = pp.tile([128, 1], BF16)
            ones_r = pp.tile([1, 128], F32)
            eps_sb = pp.tile([128, 1], F32)
            htT = pp.tile([128, DC, B], BF16)          # recurrent state
            z_sb = pp.tile([128, S, CC, B], FP16)      # Z' = x@W1_bot + b1
            xT_sb = pp.tile([128, S, DC, B], BF16)     # x feature-major
            outT_sb = pp.tile([128, S, DC, B], FP16)   # ht trajectory

            nc.gpsimd.dma_start(wi_sb, wi_d[:])
            nc.gpsimd.dma_start(w1t_sb, w1t_d[:])
            nc.gpsimd.dma_start(w1b_sb, w1b_d[:])
            nc.gpsimd.dma_start(w2_sb, w2_d[:])
            nc.gpsimd.dma_start(b1t_sb, b1t_d[:])
            nc.gpsimd.dma_start(b2f_sb, b2f_d[:])
            nc.gpsimd.dma_start(gT_sb, gT_d[:])
            nc.gpsimd.dma_start(bT_sb, bT_d[:])
            nc.gpsimd.dma_start(idb_sb, idb_d[:])
            nc.gpsimd.dma_start(idh_sb, idh_d[:])
            nc.gpsimd.dma_start(htT, h0_d[:])
            nc.gpsimd.dma_start(gbc_sb, grow_d[0:1, :].broadcast_to([128, D]))
            nc.gpsimd.dma_start(bbc_sb, brow_d[0:1, :].broadcast_to([128, D]))
            nc.gpsimd.dma_start(bibc_sb, birow_d[0:1, :].broadcast_to([128, D]))
            nc.vector.memset(ones_b, 1.0)
            nc.vector.memset(ones_r, 1.0)
            nc.vector.memset(eps_sb, LN_EPS)
            if n_steps < S:  # sim-only truncated runs: phase 3 reads all steps
                nc.vector.memset(outT_sb, 0.0)

            # ================= Phase 1: bulk init transform =================
            # x = LN((seq*mask) @ W_init + b_init) * mask, then xT and
            # Z' = x @ W1_bot + b1, both stored feature-major as [p, s, c, b].
            with (
                tc.tile_pool(name="bulk", bufs=2) as bp,
                tc.tile_pool(name="bulkps", bufs=2, space="PSUM") as bps,
                tc.tile_pool(name="bulkps2", bufs=2, space="PSUM") as bps2,
            ):
                for tt in range(NTT):
                    bb = tt // (S // 128)          # batch index of this tile
                    s0 = (tt % (S // 128)) * 128   # first step covered
                    seq_t = bp.tile([128, D + 1], BF16, tag="seq")
                    nc.gpsimd.dma_start(seq_t, seq_d[tt * 128 : (tt + 1) * 128, :])
                    mkbc = seq_t[:, D : D + 1].broadcast_to([128, D])
                    mseq = bp.tile([128, D], BF16, tag="mseq")
                    nc.vector.tensor_mul(mseq, seq_t[:, 0:D], mkbc)
                    # transpose masked seq tile -> seqT chunks
                    ps_tr = bps.tile([128, D], BF16, tag="tr")
                    for c in range(DC):
                        nc.tensor.transpose(
                            ps_tr[:, c * 128 : (c + 1) * 128],
                            mseq[:, c * 128 : (c + 1) * 128],
                            idb_sb,
                        )
                    seqT = bp.tile([128, DC, 128], BF16, tag="seqT")
                    nc.vector.tensor_copy(
                        seqT, ps_tr.rearrange("p (c t) -> p c t", c=DC)
                    )
                    # init matmul: xpre[tok, :] = (seq*mask) @ W_init
                    xpre = bps2.tile([128, D], F32, tag="xpre")
                    for kc in range(DC):
                        nc.tensor.matmul(
                            xpre,
                            seqT[:, kc, :],
                            wi_sb[:, kc, :],
                            start=(kc == 0),
                            stop=(kc == DC - 1),
                        )
                    xpb = bp.tile([128, D], F32, tag="xpb")
                    nc.vector.tensor_add(xpb, xpre, bibc_sb)  # + b_init
                    # layernorm along free dim (feature)
                    stats = bp.tile([128, 6], F32, tag="stats")
                    mv = bp.tile([128, 2], F32, tag="mv")
                    nc.vector.bn_stats(stats, xpb)
                    nc.vector.bn_aggr(mv, stats)
                    std = bp.tile([128, 1], F32, tag="std")
                    nc.scalar.activation(std, mv[:, 1:2], AF.Sqrt, bias=eps_sb)
                    inv = bp.tile([128, 1], F32, tag="inv")
                    nc.vector.reciprocal(inv, std)
                    u_sb = bp.tile([128, D], F32, tag="u")
                    nc.vector.tensor_scalar(
                        u_sb, xpb, mv[:, 0:1], inv, OP.subtract, OP.mult
                    )
                    nc.vector.tensor_mul(u_sb, u_sb, gbc_sb)
                    nc.vector.tensor_add(u_sb, u_sb, bbc_sb)
                    xb = bp.tile([128, D], BF16, tag="xb")
                    nc.vector.tensor_mul(xb, u_sb, mkbc)
                    # transpose x -> xT chunks (contiguous staging)
                    ps_tr2 = bps.tile([128, D], BF16, tag="tr")
                    for c in range(DC):
                        nc.tensor.transpose(
                            ps_tr2[:, c * 128 : (c + 1) * 128],
                            xb[:, c * 128 : (c + 1) * 128],
                            idb_sb,
                        )
                    xTt = bp.tile([128, DC, 128], BF16, tag="xTt")
                    nc.vector.tensor_copy(
                        xTt, ps_tr2.rearrange("p (c t) -> p c t", c=DC)
                    )
                    # scatter into persistent xT [p, s, c, b]
                    nc.gpsimd.tensor_copy(
                        xT_sb[:, s0 : s0 + 128, :, bb],
                        xTt.rearrange("p c t -> p t c"),
                    )
                    # Z' = x @ W1_bot + b1, feature-major
                    for cc in range(CC):
                        zps = bps2.tile([128, 128], F32, tag="zps")
                        for kc in range(DC):
                            nc.tensor.matmul(
                                zps,
                                w1b_sb[:, kc, cc * 128 : (cc + 1) * 128],
                                xTt[:, kc, :],
                                start=(kc == 0),
                                stop=(kc == DC - 1),
                            )
                        nc.vector.tensor_scalar_add(
                            z_sb[:, s0 : s0 + 128, cc, bb],
                            zps,
                            b1t_sb[:, cc : cc + 1],
                        )

            # ================= Phase 2: the recurrence =================
            with (
                tc.tile_pool(name="lp", bufs=2) as lp,
                tc.tile_pool(name="ps_i", bufs=2, space="PSUM") as ps_ip,
                tc.tile_pool(name="ps_c", bufs=2, space="PSUM") as ps_cp,
                tc.tile_pool(name="ps_s", bufs=2, space="PSUM") as ps_sp,
                tc.tile_pool(name="ps_b", bufs=2, space="PSUM") as ps_bp,
            ):

                def body(zsl, xsl, osl):
                    # ---- matmul 1: interT_pre = W1_top-tiles @ htT ----
                    ps_i = ps_ip.tile([128, CC, B], F32, tag="psi")
                    for cc in range(CC):
                        for kc in range(DC):
                            nc.tensor.matmul(
                                ps_i[:, cc, :],
                                w1t_sb[:, kc, cc * 128 : (cc + 1) * 128],
                                htT[:, kc, :],
                                start=(kc == 0),
                                stop=(kc == DC - 1),
                            )
                    # gelu(x) = x * sigmoid(2*sqrt(2/pi)*(x + 0.044715 x^3))
                    # (exact rewrite of the tanh-form gelu; Sigmoid keeps the
                    # ACT engine on one table for the whole loop)
                    g1 = lp.tile([128, CC, B], F32, tag="g1")
                    nc.vector.tensor_add(g1, ps_i, zsl)
                    gx2 = lp.tile([128, CC, B], F32, tag="gx2")
                    nc.vector.tensor_mul(gx2, g1, g1)
                    nc.vector.tensor_scalar(gx2, gx2, 0.044715, 1.0, OP.mult, OP.add)
                    ginn = lp.tile([128, CC, B], F32, tag="ginn")
                    nc.vector.tensor_mul(ginn, gx2, g1)
                    gsg = lp.tile([128, CC, B], F32, tag="gsg")
                    nc.scalar.activation(
                        gsg, ginn, AF.Sigmoid, scale=float(2.0 * np.sqrt(2.0 / np.pi))
                    )
                    interT = lp.tile([128, CC, B], BF16, tag="interT")
                    nc.vector.tensor_mul(interT, g1, gsg)

                    # ---- matmul 2: contsT = W2-tiles @ interT ----
                    ps_c = ps_cp.tile([128, FC, B], F32, tag="psc")
                    for pc in range(FC):
                        for kc in range(CC):
                            nc.tensor.matmul(
                                ps_c[:, pc, :],
                                w2_sb[:, kc, pc * 128 : (pc + 1) * 128],
                                interT[:, kc, :],
                                start=(kc == 0),
                                stop=(kc == CC - 1),
                            )
                    cb = lp.tile([128, FC, B], F32, tag="cb")
                    nc.vector.tensor_add(cb, ps_c, b2f_sb)
                    gate = lp.tile([128, 3 * DC, B], F32, tag="gate")
                    nc.scalar.activation(gate, cb[:, 0 : 3 * DC, :], AF.Sigmoid)

                    # ---- gated combine: t = f1*ht + f2*x + i*parent ----
                    t1 = lp.tile([128, DC, B], F32, tag="t1")
                    nc.vector.tensor_mul(t1, gate[:, 0:DC, :], htT)
                    t2 = lp.tile([128, DC, B], F32, tag="t2")
                    nc.vector.tensor_mul(t2, gate[:, DC : 2 * DC, :], xsl)
                    nc.vector.tensor_add(t1, t1, t2)
                    t3 = lp.tile([128, DC, B], F32, tag="t3")
                    nc.vector.tensor_mul(
                        t3, gate[:, 2 * DC : 3 * DC, :], cb[:, 3 * DC : 4 * DC, :]
                    )
                    tb = lp.tile([128, DC, B], BF16, tag="tb")
                    nc.vector.tensor_add(tb, t1, t3)
                    tsq = lp.tile([128, DC, B], BF16, tag="tsq")
                    nc.vector.tensor_mul(tsq, tb, tb)

                    # ---- LN stats across partitions via ones-matmul ----
                    ps_s = ps_sp.tile([1, 2, DC * B], F32, tag="pss")
                    nc.tensor.matmul(
                        ps_s[0:1, 0, :],
                        ones_b,
                        tb.rearrange("p c b -> p (c b)"),
                        start=True,
                        stop=True,
                    )
                    nc.tensor.matmul(
                        ps_s[0:1, 1, :],
                        ones_b,
                        tsq.rearrange("p c b -> p (c b)"),
                        start=True,
                        stop=True,
                    )
                    mrow = lp.tile([1, 2, B], F32, tag="mrow")
                    nc.vector.reduce_sum(
                        mrow,
                        ps_s.rearrange("p t (c b) -> p t b c", c=DC),
                        axis=mybir.AxisListType.X,
                    )
                    nc.vector.tensor_scalar_mul(mrow, mrow, 1.0 / D)
                    m2 = lp.tile([1, B], F32, tag="m2")
                    nc.vector.tensor_mul(m2, mrow[:, 0, :], mrow[:, 0, :])
                    var = lp.tile([1, B], F32, tag="var")
                    nc.vector.tensor_sub(var, mrow[:, 1, :], m2)
                    # inv = 1/sqrt(var+eps) via bit-trick + 2 Newton steps
                    # (avoids the Sqrt ACT table, keeping ACT on Sigmoid)
                    vh = lp.tile([1, B], F32, tag="vh")
                    nc.vector.tensor_scalar(
                        vh, var, 0.5, LN_EPS * 0.5, OP.mult, OP.add
                    )
                    vf = lp.tile([1, B], F32, tag="vf")
                    nc.vector.tensor_scalar_add(vf, var, LN_EPS)
                    yi = lp.tile([1, B], mybir.dt.int32, tag="yi")
                    nc.vector.tensor_scalar(
                        yi, vf.bitcast(mybir.dt.int32), 1, -1,
                        OP.arith_shift_right, OP.bitwise_xor,
                    )
                    nc.vector.tensor_scalar_add(yi, yi, 0x5F3759DF + 1)
                    y = yi.bitcast(F32)
                    for _ in range(2):
                        ya = lp.tile([1, B], F32, tag="ya")
                        nc.vector.tensor_mul(ya, y, y)
                        nc.vector.tensor_mul(ya, ya, vh)
                        nc.vector.tensor_scalar(ya, ya, -1.0, 1.5, OP.mult, OP.add)
                        nc.vector.tensor_mul(y, y, ya)
                    nc.vector.tensor_copy(mrow[:, 1, :], y)

                    # ---- broadcast mean/inv across partitions (K=1 matmul) ----
                    ps_b = ps_bp.tile([128, 2, B], F32, tag="psb")
                    nc.tensor.matmul(
                        ps_b.rearrange("p t b -> p (t b)"),
                        ones_r,
                        mrow.rearrange("p t b -> p (t b)"),
                        start=True,
                        stop=True,
                    )
                    u = lp.tile([128, DC, B], F32, tag="u")
                    nc.vector.tensor_sub(
                        u, tb, ps_b[:, 0:1, :].broadcast_to([128, DC, B])
                    )
                    nc.vector.tensor_mul(
                        u, u, ps_b[:, 1:2, :].broadcast_to([128, DC, B])
                    )
                    nc.vector.tensor_mul(
                        u, u, gT_sb.unsqueeze(2).broadcast_to([128, DC, B])
                    )
                    nc.vector.tensor_add(
                        htT, u, bT_sb.unsqueeze(2).broadcast_to([128, DC, B])
                    )
                    nc.gpsimd.tensor_copy(osl, htT.unsqueeze(1))  # osl static

                SPB = 8  # steps per dynamic block.  Only 3 instructions per
                # block touch a dynamically-offset AP (engine registers are
                # scarce); per-step accesses go through static ring tiles.

                def block_body(s0):
                    zblk = lp.tile([128, SPB, CC, B], FP16, tag="zblk")
                    nc.vector.tensor_copy(zblk, z_sb[:, ds(s0, SPB), :, :])
                    xblk = lp.tile([128, SPB, DC, B], BF16, tag="xblk")
                    nc.vector.tensor_copy(xblk, xT_sb[:, ds(s0, SPB), :, :])
                    oblk = lp.tile([128, SPB, DC, B], FP16, tag="oblk")
                    for j in range(SPB):
                        body(
                            zblk[:, j, :, :],
                            xblk[:, j, :, :],
                            oblk[:, j : j + 1, :, :],
                        )
                    nc.gpsimd.tensor_copy(outT_sb[:, ds(s0, SPB), :, :], oblk)

                assert n_steps % SPB == 0
                tc.For_i_unrolled(0, n_steps, SPB, block_body, max_unroll=unroll)

            # ================= Phase 3: emit output =================
            with (
                tc.tile_pool(name="ops", bufs=4, space="PSUM") as ops_p,
                tc.tile_pool(name="ostage", bufs=4) as ost_p,
            ):
                for tt in range(NTT):
                    bb = tt // (S // 128)
                    s0 = (tt % (S // 128)) * 128
                    for c in range(DC):
                        po = ops_p.tile([128, 128], FP16, tag="po")
                        nc.tensor.transpose(
                            po, outT_sb[:, s0 : s0 + 128, c, bb], idh_sb
                        )
                        og = ost_p.tile([128, 128], FP16, tag="og")
                        nc.vector.tensor_copy(og, po)
                        nc.sync.dma_start(
                            out_d[
                                tt * 128 : (tt + 1) * 128, c * 128 : (c + 1) * 128
                            ],
                            og,
                        )

    nc.finalize()
    return nc


# ======================= host side =======================

def _prep_core_inputs(inputs, core):
    sl = slice(core * B, (core + 1) * B)
    seq = np.asarray(inputs["sequence"])[sl].reshape(TOK, D)
    mask = np.asarray(inputs["input_mask"])[sl].reshape(TOK, 1)
    seqm = np.concatenate(
        [seq.astype(ml_dtypes.bfloat16), mask.astype(ml_dtypes.bfloat16)], axis=1
    )
    return {"seq": np.ascontiguousarray(seqm)}


def _prep_shared_inputs(inputs):
    bf = ml_dtypes.bfloat16
    W_init = np.asarray(inputs["W_init"], np.float32)
    b_init = np.asarray(inputs["b_init"], np.float32)
    W1 = np.asarray(inputs["W1"], np.float32)
    b1 = np.asarray(inputs["b1"], np.float32)
    W2 = np.asarray(inputs["W2"], np.float32)
    b2 = np.asarray(inputs["b2"], np.float32)
    g = np.asarray(inputs["ln_g"], np.float32)
    bb = np.asarray(inputs["ln_b"], np.float32)
    START = np.asarray(inputs["START"], np.float32)
    return {
        "wi": np.ascontiguousarray(
            W_init.reshape(DC, 128, D).transpose(1, 0, 2)
        ).astype(bf),
        "w1t": np.ascontiguousarray(
            W1[:D].reshape(DC, 128, C).transpose(1, 0, 2)
        ).astype(bf),
        "w1b": np.ascontiguousarray(
            W1[D:].reshape(DC, 128, C).transpose(1, 0, 2)
        ).astype(bf),
        "w2": np.ascontiguousarray(
            W2.reshape(CC, 128, 4 * D).transpose(1, 0, 2)
        ).astype(bf),
        "b1t": np.ascontiguousarray(b1.reshape(CC, 128).T).astype(np.float32),
        "b2f": np.ascontiguousarray(
            np.repeat(b2.reshape(FC, 128).T[:, :, None], B, axis=2)
        ).astype(np.float32),
        "gT": np.ascontiguousarray(g.reshape(DC, 128).T).astype(np.float32),
        "bT": np.ascontiguousarray(bb.reshape(DC, 128).T).astype(np.float32),
        "grow": g.reshape(1, D).astype(bf),
        "brow": bb.reshape(1, D).astype(bf),
        "birow": b_init.reshape(1, D).astype(bf),
        "h0": np.ascontiguousarray(
            np.repeat(START.reshape(DC, 128).T[:, :, None], B, axis=2)
        ).astype(bf),
        "idb": np.eye(128, dtype=bf),
        "idh": np.eye(128, dtype=np.float16),
    }


def _postprocess(core_outs, inputs):
    mask = np.asarray(inputs["input_mask"], np.float32)
    START = np.asarray(inputs["START"], np.float32)
    out_raw = np.concatenate(
        [np.asarray(o).astype(np.float32).reshape(B, S, D) for o in core_outs],
        axis=0,
    )
    out_seq = out_raw * mask[..., None]
    gs = np.broadcast_to(START, (N, D)).astype(np.float32).copy()
    any_mask = mask.sum(axis=1) > 0
    last_idx = np.where(
        any_mask, S - 1 - np.argmax(mask[:, ::-1] > 0, axis=1), 0
    )
    gs[any_mask] = out_raw[np.arange(N)[any_mask], last_idx[any_mask]]
    return out_seq.astype(np.float32), gs.astype(np.float32)


def kernel(**inputs):
    from concourse.bass_utils import run_bass_kernel_spmd

    nc = build()
    shared = _prep_shared_inputs(inputs)
    in_maps = []
    for core in range(NCORES):
        m = dict(shared)
        m.update(_prep_core_inputs(inputs, core))
        in_maps.append(m)
    res = run_bass_kernel_spmd(nc, in_maps, core_ids=list(range(NCORES)))
    return _postprocess([r["out"] for r in res.results], inputs)
